# revision 1
# baseline (speedup 1.0000x reference)
"""Trainium2 Bass kernel for the autoregressive pointer-attention decoder.

Full inputs -> shard batch over 8 NeuronCores -> Bass/Tile kernel -> full output.
Self-contained: only needs /opt/trn_rl_repo (part of the container image).
"""
import sys

sys.path.insert(0, "/opt/trn_rl_repo")

import numpy as np

import concourse.bass as bass
import concourse.mybir as mybir
import concourse.tile as tile
from concourse.bass_utils import run_bass_kernel_spmd
from bass_rust import SyncInfo

# Problem shapes (hardcoded per spec)
B, T, N, D, H, DH = 32, 512, 512, 256, 8, 32
N_CORES = 8
B_LOC = B // N_CORES  # 4 batch elements per core

F32 = mybir.dt.float32
F16 = mybir.dt.float16
I32 = mybir.dt.int32
U8 = mybir.dt.uint8
AF = mybir.ActivationFunctionType
ALU = mybir.AluOpType

SCALE_DH = 1.0 / np.sqrt(DH)
INV_N = 1.0 / N
CLIP_C = 2048.0          # masked-exp offset; exp(-2048) == 0 in fp32
BIG = 1e9 - CLIP_C       # so that masked output lands at -1e9 - L

# ln(m) ~ LC5*m^5 + ... + LC1*m + LC0 on [1, 2), max err 2.2e-5
LC = [-1.9316715417211114, 3.4982279012105137, -2.420812563220031,
      1.1048082362000629, -0.28063254044994346, 0.030102625011718297]
LN2 = 0.6931471805599453

# tuning knobs (read at build time)
import os
MASK_ENG = os.environ.get("K_MASK_ENG", "dve")      # dve | pool | alt
MASKT_MODE = os.environ.get("K_MASKT", "pe")        # pe | xbar


def _split_excess_waits(nc, max_waits=1):
    """walrus in this container rejects >1 sync-wait per instruction; move
    excess waits to preceding same-engine EventSemaphore instructions."""
    nid = [0]
    for f in nc.m.functions:
        for blk in f.blocks:
            new_insts = []
            for inst in blk.instructions:
                si = inst.sync_info
                waits = list(si.on_wait) if si is not None else []
                if len(waits) > max_waits:
                    keep, excess = waits[:max_waits], waits[max_waits:]
                    for w in excess:
                        nid[0] += 1
                        esem = mybir.InstEventSemaphore(
                            name=f"waitfix-{nid[0]}",
                            ins=[], outs=[],
                            sync_info=SyncInfo(on_wait=[w], on_update=[]),
                        )
                        esem.engine = inst.engine
                        new_insts.append(esem)
                    inst.sync_info = SyncInfo(on_wait=keep,
                                              on_update=list(si.on_update))
                new_insts.append(inst)
            blk.instructions[:] = new_insts


def build_kernel(b_loc=B_LOC, num_devices=N_CORES, waitfix=True, repeat=1):
    nc = bass.Bass("TRN2", target_bir_lowering=False, debug=False,
                   num_devices=num_devices)

    emb_d = nc.dram_tensor("embeddings", [b_loc, N, D], F16, kind="ExternalInput").ap()
    nodes_d = nc.dram_tensor("current_nodes", [b_loc, T], F16, kind="ExternalInput").ap()
    mask_d = nc.dram_tensor("action_mask", [b_loc, T, N], U8, kind="ExternalInput").ap()
    wnode_d = nc.dram_tensor("W_node", [D, 3 * D], F16, kind="ExternalInput").ap()
    wfix_d = nc.dram_tensor("W_fixed", [D, D], F16, kind="ExternalInput").ap()
    wstep_d = nc.dram_tensor("W_step", [D, D], F16, kind="ExternalInput").ap()
    wout_d = nc.dram_tensor("W_out", [D, D], F16, kind="ExternalInput").ap()
    out_d = nc.dram_tensor("log_p", [b_loc, T, N], F32, kind="ExternalOutput").ap()

    with tile.TileContext(nc) as tc:
        _build_body(nc, tc, b_loc, emb_d, nodes_d, mask_d, wnode_d, wfix_d,
                    wstep_d, wout_d, out_d, repeat=repeat)

    if waitfix:
        _split_excess_waits(nc)
    return nc


def _build_body(nc, tc, b_loc, emb_d, nodes_d, mask_d, wnode_d, wfix_d,
                wstep_d, wout_d, out_d, repeat=1):
    from contextlib import ExitStack
    ctx = ExitStack()
    with ctx:
        consts = ctx.enter_context(tc.tile_pool(name="consts", bufs=1))
        wpool = ctx.enter_context(tc.tile_pool(name="wpool", bufs=1))
        sb = ctx.enter_context(tc.tile_pool(name="sb", bufs=2))
        # PSUM budget (8 banks): pbig 4 (scores / logits, shared tag) +
        # pacc 2 (heads, sums accumulators) + pmisc 2 (everything else)
        ps = ctx.enter_context(tc.tile_pool(name="ps", bufs=1, space="PSUM"))
        dram = ctx.enter_context(tc.tile_pool(name="dram", bufs=2, space="DRAM"))

        def ptile(shape, tag, bufs):
            return ps.tile(shape, F32, tag=tag, bufs=bufs, name=tag)

        # ---- constants ----
        ident = consts.tile([128, 128], F16)
        nc.gpsimd.memset(ident, 0.0)
        nc.gpsimd.affine_select(out=ident, in_=ident, compare_op=ALU.not_equal,
                                fill=1.0, base=0, pattern=[[-1, 128]],
                                channel_multiplier=1)
        ones_blk = consts.tile([128, 32], F16)
        nc.gpsimd.memset(ones_blk, 1.0)
        meanones = consts.tile([128, 1], F16)
        nc.gpsimd.memset(meanones, INV_N)
        negc = consts.tile([128, 1], F32)
        nc.gpsimd.memset(negc, -CLIP_C)
        pidx_i = consts.tile([128, 4], I32)
        for nt in range(4):
            nc.gpsimd.iota(pidx_i[:, nt:nt + 1], pattern=[[0, 1]], base=128 * nt,
                           channel_multiplier=1)
        pidx_h = consts.tile([128, 4], F16)
        nc.vector.tensor_copy(pidx_h, pidx_i)

        # ---- weights to SBUF ----
        wnode_sb = wpool.tile([128, 2, 3 * D], F16)
        nc.sync.dma_start(wnode_sb, wnode_d.rearrange("(c p) j -> p c j", p=128))
        wfix_sb = wpool.tile([128, 2, D], F16)
        nc.sync.dma_start(wfix_sb, wfix_d.rearrange("(c p) j -> p c j", p=128))
        wstep_sb = wpool.tile([128, 2, D], F16)
        nc.sync.dma_start(wstep_sb, wstep_d.rearrange("(c p) j -> p c j", p=128))
        wout_sb = wpool.tile([128, 2, D], F16)
        nc.sync.dma_start(wout_sb, wout_d.rearrange("(c p) j -> p c j", p=128))

        for b in [bb for _ in range(repeat) for bb in range(b_loc)]:
            # ---- load batch data ----
            emb_nat = sb.tile([128, 4, D], F16, tag="emb_nat")
            nc.sync.dma_start(emb_nat, emb_d[b].rearrange("(o p) d -> p o d", p=128))
            c_bcast = sb.tile([128, T], F16, tag="c_bcast", bufs=2)
            nc.sync.dma_start(c_bcast, nodes_d[b][None, :].to_broadcast((128, T)))

            mask_u8 = sb.tile([128, 4, N], U8, tag="mask_u8", bufs=2)
            nc.sync.dma_start(mask_u8, mask_d[b].rearrange("(o p) n -> p o n", p=128))
            mask01 = sb.tile([128, 4, N], F16, tag="mask01", bufs=2)
            nc.vector.tensor_copy(mask01, mask_u8)
            invm = sb.tile([128, 4, N], F16, tag="invm", bufs=2)
            nc.vector.tensor_scalar(invm, mask_u8, -1.0, 1.0, ALU.mult, ALU.add)

            maskT = sb.tile([128, 4, T], F16, tag="maskT", bufs=2)
            if MASKT_MODE == "xbar":
                mstage = dram.tile([T, N], F16, tag="mstage")
                nc.sync.dma_start(mstage.rearrange("(o p) n -> p o n", p=128), mask01)
                for ntt in range(4):
                    nc.sync.dma_start_transpose(maskT[:, ntt, :],
                                                mstage[:, ntt * 128:(ntt + 1) * 128])
            else:
                for ntt in range(4):
                    ptm = ptile([128, 512], "pmisc", 2)
                    for to in range(4):
                        nc.tensor.matmul(ptm[:, to * 128:(to + 1) * 128],
                                         mask01[:, to, ntt * 128:(ntt + 1) * 128],
                                         ident, start=True, stop=True)
                    nc.vector.tensor_copy(maskT[:, ntt, :], ptm)

            # one-hot gather matrix S[n, t] = (n == current_nodes[t])
            s_oh = sb.tile([128, 4, T], F16, tag="s_oh", bufs=2)
            for ntt in range(4):
                nc.vector.tensor_tensor(
                    s_oh[:, ntt, :], c_bcast,
                    pidx_h[:, ntt:ntt + 1].to_broadcast((128, T)), ALU.is_equal)

            # ---- embT via PE (identity matmul), one psum tile per d-chunk ----
            embT = sb.tile([128, 2, N], F16, tag="embT", bufs=2)
            for dc in range(2):
                pt = ptile([128, 512], "pmisc", 2)
                for no in range(4):
                    nc.tensor.matmul(pt[:, no * 128:(no + 1) * 128],
                                     emb_nat[:, no, dc * 128:(dc + 1) * 128],
                                     ident, start=True, stop=True)
                nc.vector.tensor_copy(embT[:, dc, :], pt)

            # ---- graph context: gcT = W_fixed^T @ mean(emb) ----
            p_mean = ptile([128, 2], "pmisc", 2)
            for dc in range(2):
                for no in range(4):
                    nc.tensor.matmul(p_mean[:, dc:dc + 1],
                                     emb_nat[:, no, dc * 128:(dc + 1) * 128],
                                     meanones, start=(no == 0), stop=(no == 3))
            meanT = sb.tile([128, 2], F16, tag="meanT")
            nc.vector.tensor_copy(meanT, p_mean)
            p_gc = ptile([128, 2], "pmisc", 2)
            for dpc in range(2):
                for dc in range(2):
                    nc.tensor.matmul(p_gc[:, dpc:dpc + 1],
                                     wfix_sb[:, dc, dpc * 128:(dpc + 1) * 128],
                                     meanT[:, dc:dc + 1],
                                     start=(dc == 0), stop=(dc == 1))
            gcT = sb.tile([128, 2], F32, tag="gcT")
            nc.vector.tensor_copy(gcT, p_gc)

            # ---- kT / logit_kT: projT[j, n] = W^T emb^T ----
            kT = sb.tile([128, 2, N], F16, tag="kT", bufs=2)
            lkT = sb.tile([128, 2, N], F16, tag="lkT", bufs=2)
            for j, dst, jj in ((0, kT, 0), (1, kT, 1), (4, lkT, 0), (5, lkT, 1)):
                pp = ptile([128, N], "pmisc", 2)
                for dc in range(2):
                    nc.tensor.matmul(pp, wnode_sb[:, dc, j * 128:(j + 1) * 128],
                                     embT[:, dc, :], start=(dc == 0), stop=(dc == 1))
                nc.vector.tensor_copy(dst[:, jj, :], pp)

            # ---- v (natural layout, fp16) ----
            v_sb = sb.tile([128, 4, D], F16, tag="v_sb", bufs=2)
            for no in range(4):
                pv = ptile([128, D], "pmisc", 2)
                for dc in range(2):
                    nc.tensor.matmul(pv, embT[:, dc, no * 128:(no + 1) * 128],
                                     wnode_sb[:, dc, D:2 * D],
                                     start=(dc == 0), stop=(dc == 1))
                nc.vector.tensor_copy(v_sb[:, no, :], pv)

            # ---- gathered step embeddings, transposed: seT = emb^T @ S ----
            seT = sb.tile([128, 2, T], F16, tag="seT", bufs=2)
            for dc in range(2):
                pse = ptile([128, T], "pmisc", 2)
                for no in range(4):
                    nc.tensor.matmul(pse, emb_nat[:, no, dc * 128:(dc + 1) * 128],
                                     s_oh[:, no, :], start=(no == 0), stop=(no == 3))
                nc.vector.tensor_copy(seT[:, dc, :], pse)

            # ---- qT = W_step^T @ seT + gcT ----
            qT = sb.tile([128, 2, T], F16, tag="qT", bufs=2)
            for dpc in range(2):
                pq = ptile([128, T], "pmisc", 2)
                for dc in range(2):
                    nc.tensor.matmul(pq, wstep_sb[:, dc, dpc * 128:(dpc + 1) * 128],
                                     seT[:, dc, :], start=(dc == 0), stop=(dc == 1))
                nc.vector.tensor_scalar(qT[:, dpc, :], pq, gcT[:, dpc:dpc + 1],
                                        None, ALU.add)

            # ---- multi-head attention, 2 groups of 4 heads, 2-head waves ----
            hn = sb.tile([128, 2, T], F16, tag="hn", bufs=2)  # normalized headsT
            for g in range(2):
                e_sb = sb.tile([128, 4, 4, T], F16, tag="e_sb", bufs=2)
                p_hd = ptile([128, T], "pacc", 2)
                p_sm = ptile([128, T], "pacc", 2)
                for nt in range(4):
                    for half in range(2):
                        p_sc = ptile([128, 2, T], "pbig", 2)
                        for hh2 in range(2):
                            hh = 2 * half + hh2
                            nc.tensor.matmul(
                                p_sc[:, hh2, :],
                                kT[hh * 32:(hh + 1) * 32, g, nt * 128:(nt + 1) * 128],
                                qT[hh * 32:(hh + 1) * 32, g, :],
                                start=True, stop=True, tile_position=(hh * 32, 0))
                        esl = e_sb[:, nt, 2 * half:2 * half + 2, :]
                        nc.scalar.activation(esl, p_sc, AF.Exp, scale=SCALE_DH)
                        if MASK_ENG == "pool":
                            meng = nc.gpsimd
                        elif MASK_ENG == "alt":
                            meng = nc.gpsimd if (2 * nt + half) % 2 else nc.vector
                        else:
                            meng = nc.vector
                        meng.tensor_tensor(
                            esl, esl,
                            maskT[:, nt, None, :].to_broadcast((128, 2, T)), ALU.mult)
                        for hh2 in range(2):
                            hh = 2 * half + hh2
                            h = 4 * g + hh
                            nc.tensor.matmul(
                                p_hd[hh * 32:(hh + 1) * 32, :],
                                v_sb[:, nt, h * 32:(h + 1) * 32],
                                e_sb[:, nt, hh, :],
                                start=(nt == 0), stop=(nt == 3),
                                tile_position=(0, hh * 32), skip_group_check=True)
                            nc.tensor.matmul(
                                p_sm[hh * 32:(hh + 1) * 32, :],
                                ones_blk,
                                e_sb[:, nt, hh, :],
                                start=(nt == 0), stop=(nt == 3),
                                tile_position=(0, hh * 32), skip_group_check=True)
                recip = sb.tile([128, T], F32, tag="recip", bufs=2)
                nc.vector.reciprocal(out=recip, in_=p_sm)
                nc.vector.tensor_tensor(hn[:, g, :], p_hd, recip, ALU.mult)

            # ---- glimpseT = W_out^T @ hn ----
            gT = sb.tile([128, 2, T], F16, tag="gT", bufs=2)
            for dpc in range(2):
                pg = ptile([128, T], "pmisc", 2)
                for dc in range(2):
                    nc.tensor.matmul(pg, wout_sb[:, dc, dpc * 128:(dpc + 1) * 128],
                                     hn[:, dc, :], start=(dc == 0), stop=(dc == 1))
                nc.vector.tensor_copy(gT[:, dpc, :], pg)

            # ---- pointer logits [t, n] ----
            t1 = sb.tile([128, 4, N], F32, tag="t1", bufs=2)
            for tg in range(2):
                p_lg = ptile([128, 2, N], "pbig", 2)
                for tc2 in range(2):
                    tcn = 2 * tg + tc2
                    for dc in range(2):
                        nc.tensor.matmul(p_lg[:, tc2, :],
                                         gT[:, dc, tcn * 128:(tcn + 1) * 128],
                                         lkT[:, dc, :], start=(dc == 0), stop=(dc == 1))
                nc.scalar.activation(t1[:, 2 * tg:2 * tg + 2, :], p_lg, AF.Tanh,
                                     scale=1.0 / np.sqrt(D))

            # ein = (t1 + C/10) * m; exp(10*ein - C): masked -> 0, else e^{10 t1}
            ein = sb.tile([128, 4, N], F32, tag="ein", bufs=2)
            nc.vector.scalar_tensor_tensor(ein, t1, CLIP_C / 10.0, mask01,
                                           ALU.add, ALU.mult)
            s2 = sb.tile([128, 4], F32, tag="s2", bufs=2)
            e2scr = sb.tile([128, N], F16, tag="e2scr")
            for tcn in range(4):
                nc.scalar.activation(e2scr, ein[:, tcn, :], AF.Exp, scale=10.0,
                                     bias=negc[:, 0:1],
                                     accum_out=s2[:, tcn:tcn + 1])

            # invmB = invm*(BIG/10) - ein  (masked -> BIG/10, else -ein)
            invmB = sb.tile([128, 4, N], F32, tag="invmB", bufs=2)
            nc.vector.scalar_tensor_tensor(invmB, invm, BIG / 10.0, ein,
                                           ALU.mult, ALU.subtract)

            # ---- Lp = (ln(s2) + C)/10 on DVE (bit trick + poly) ----
            s2i = s2.bitcast(I32)
            e_i = sb.tile([128, 4], I32, tag="e_i", bufs=2)
            nc.vector.tensor_scalar(e_i, s2i, 23, None, ALU.arith_shift_right)
            e_f = sb.tile([128, 4], F32, tag="e_f", bufs=2)
            nc.vector.tensor_scalar(e_f, e_i, -127, LN2 / 10.0, ALU.add, ALU.mult)
            m_i = sb.tile([128, 4], I32, tag="m_i", bufs=2)
            nc.vector.tensor_scalar(m_i, s2i, 0x007FFFFF, 0x3F800000,
                                    ALU.bitwise_and, ALU.bitwise_or)
            m_f = m_i.bitcast(F32)
            q5 = sb.tile([128, 4], F32, tag="q5", bufs=2)
            nc.vector.scalar_tensor_tensor(q5, m_f, LC[4] / LC[5], m_f,
                                           ALU.add, ALU.mult)
            nc.vector.scalar_tensor_tensor(q5, q5, LC[3] / LC[5], m_f,
                                           ALU.add, ALU.mult)
            nc.vector.scalar_tensor_tensor(q5, q5, LC[2] / LC[5], m_f,
                                           ALU.add, ALU.mult)
            nc.vector.scalar_tensor_tensor(q5, q5, LC[1] / LC[5], m_f,
                                           ALU.add, ALU.mult)
            lp = sb.tile([128, 4], F32, tag="lp", bufs=2)
            nc.vector.scalar_tensor_tensor(lp, q5, LC[5] / 10.0, e_f,
                                           ALU.mult, ALU.add)
            nc.vector.tensor_scalar(lp, lp, (LC[0] + CLIP_C) / 10.0, None, ALU.add)

            # ---- final: out = -10*(invmB + Lp) ----
            for tcn in range(4):
                o = sb.tile([128, N], F32, tag="o_out")
                nc.vector.tensor_scalar(o, invmB[:, tcn, :],
                                        lp[:, tcn:tcn + 1], -10.0,
                                        ALU.add, ALU.mult)
                nc.sync.dma_start(out_d[b, tcn * 128:(tcn + 1) * 128, :], o)


_NC_CACHE = {}


def kernel(**inputs):
    emb = np.ascontiguousarray(np.asarray(inputs["embeddings"]).astype(np.float16))
    nodes = np.ascontiguousarray(np.asarray(inputs["current_nodes"]).astype(np.float16))
    mask = np.ascontiguousarray(np.asarray(inputs["action_mask"]).astype(np.uint8))
    w_node = np.ascontiguousarray(np.asarray(inputs["W_node"]).astype(np.float16))
    w_fixed = np.ascontiguousarray(np.asarray(inputs["W_fixed"]).astype(np.float16))
    w_step = np.ascontiguousarray(np.asarray(inputs["W_step"]).astype(np.float16))
    w_out = np.ascontiguousarray(np.asarray(inputs["W_out"]).astype(np.float16))

    if "nc" not in _NC_CACHE:
        _NC_CACHE["nc"] = build_kernel()
    nc = _NC_CACHE["nc"]

    in_maps = []
    for c in range(N_CORES):
        lo, hi = c * B_LOC, (c + 1) * B_LOC
        in_maps.append({
            "embeddings": emb[lo:hi],
            "current_nodes": nodes[lo:hi],
            "action_mask": mask[lo:hi],
            "W_node": w_node,
            "W_fixed": w_fixed,
            "W_step": w_step,
            "W_out": w_out,
        })
    res = run_bass_kernel_spmd(nc, in_maps, list(range(N_CORES)))
    out = np.concatenate([res.results[c]["log_p"] for c in range(N_CORES)], axis=0)
    return out.astype(np.float32)


if __name__ == "__main__":
    import reference
    inputs = {k: np.asarray(v) for k, v in reference.setup_inputs().items()}
    expected = np.asarray(reference.reference(**inputs))
    actual = kernel(**inputs)
    err = np.abs(actual - expected)
    denom = np.maximum(np.abs(expected), 1e-6)
    print("max abs err:", err.max())
    print("max rel err:", (err / denom).max())



# revision 18
# speedup vs baseline: 113.4418x; 113.4418x over previous
"""Trainium2 Bass kernel for the autoregressive pointer-attention decoder.

Full inputs -> shard batch over 8 NeuronCores -> Bass/Tile kernel -> full output.
Self-contained: only needs /opt/trn_rl_repo (part of the container image).
"""
import sys

sys.path.insert(0, "/opt/trn_rl_repo")

import numpy as np

import concourse.bass as bass
import concourse.mybir as mybir
import concourse.tile as tile
from concourse.bass_utils import run_bass_kernel_spmd
from bass_rust import SyncInfo

# Problem shapes (hardcoded per spec)
B, T, N, D, H, DH = 32, 512, 512, 256, 8, 32
N_CORES = 8
B_LOC = B // N_CORES  # 4 batch elements per core

F32 = mybir.dt.float32
F16 = mybir.dt.float16
I32 = mybir.dt.int32
U8 = mybir.dt.uint8
AF = mybir.ActivationFunctionType
ALU = mybir.AluOpType

SCALE_DH = 1.0 / np.sqrt(DH)
INV_N = 1.0 / N
CLIP_C = 2048.0          # masked-exp offset; exp(-2048) == 0 in fp32
BIG = 1e9 - CLIP_C       # so that masked output lands at -1e9 - L

# ln(m) ~ LC5*m^5 + ... + LC1*m + LC0 on [1, 2), max err 2.2e-5
LC = [-1.9316715417211114, 3.4982279012105137, -2.420812563220031,
      1.1048082362000629, -0.28063254044994346, 0.030102625011718297]
LN2 = 0.6931471805599453

# tuning knobs (read at build time)
import os
MASK_MODE = os.environ.get("K_MASK", "dve")         # dve | pe
MCLIP = 240.0
RECIP_MODE = os.environ.get("K_RECIP", "exact")  # exact | approx | approx_sbuf | pool
MGEN_ENG = os.environ.get("K_MGEN", "dve")       # dve (pool: slow u8)
SOH_ENG = os.environ.get("K_SOH", "dve")         # dve (pool: unsupported)


def _split_excess_waits(nc, max_waits=1):
    """walrus in this container rejects >1 sync-wait per instruction; move
    excess waits to preceding same-engine EventSemaphore instructions."""
    nid = [0]
    for f in nc.m.functions:
        for blk in f.blocks:
            new_insts = []
            for inst in blk.instructions:
                si = inst.sync_info
                waits = list(si.on_wait) if si is not None else []
                if len(waits) > max_waits:
                    keep, excess = waits[:max_waits], waits[max_waits:]
                    for w in excess:
                        nid[0] += 1
                        esem = mybir.InstEventSemaphore(
                            name=f"waitfix-{nid[0]}",
                            ins=[], outs=[],
                            sync_info=SyncInfo(on_wait=[w], on_update=[]),
                        )
                        esem.engine = inst.engine
                        new_insts.append(esem)
                    inst.sync_info = SyncInfo(on_wait=keep,
                                              on_update=list(si.on_update))
                new_insts.append(inst)
            blk.instructions[:] = new_insts


def build_kernel(b_loc=B_LOC, num_devices=N_CORES, waitfix=True, repeat=1):
    nc = bass.Bass("TRN2", target_bir_lowering=False, debug=False,
                   num_devices=num_devices)

    emb_d = nc.dram_tensor("embeddings", [b_loc, N, D], F16, kind="ExternalInput").ap()
    nodes_d = nc.dram_tensor("current_nodes", [b_loc, T], F16, kind="ExternalInput").ap()
    mask_d = nc.dram_tensor("action_mask", [b_loc, T, N], U8, kind="ExternalInput").ap()
    wnode_d = nc.dram_tensor("W_node", [D, 3 * D], F16, kind="ExternalInput").ap()
    wfix_d = nc.dram_tensor("W_fixed", [D, D], F16, kind="ExternalInput").ap()
    wstep_d = nc.dram_tensor("W_step", [D, D], F16, kind="ExternalInput").ap()
    wout_d = nc.dram_tensor("W_out", [D, D], F16, kind="ExternalInput").ap()
    out_d = nc.dram_tensor("log_p", [b_loc, T, N], F32, kind="ExternalOutput").ap()

    with tile.TileContext(nc) as tc:
        _build_body(nc, tc, b_loc, emb_d, nodes_d, mask_d, wnode_d, wfix_d,
                    wstep_d, wout_d, out_d, repeat=repeat)

    if waitfix:
        _split_excess_waits(nc)
    return nc


def _build_body(nc, tc, b_loc, emb_d, nodes_d, mask_d, wnode_d, wfix_d,
                wstep_d, wout_d, out_d, repeat=1):
    from contextlib import ExitStack
    ctx = ExitStack()
    with ctx:
        consts = ctx.enter_context(tc.tile_pool(name="consts", bufs=1))
        wpool = ctx.enter_context(tc.tile_pool(name="wpool", bufs=1))
        sb = ctx.enter_context(tc.tile_pool(name="sb", bufs=2))
        # PSUM budget (8 banks): pbig 4 (scores / logits, shared tag) +
        # pacc 2 (heads, sums accumulators) + pmisc 2 (everything else)
        ps = ctx.enter_context(tc.tile_pool(name="ps", bufs=1, space="PSUM"))
        dram = ctx.enter_context(tc.tile_pool(name="dram", bufs=2, space="DRAM"))

        def ptile(shape, tag, bufs):
            return ps.tile(shape, F32, tag=tag, bufs=bufs, name=tag)

        def p2():
            # attention scores / logits: independent 2-bank rotation (4 banks)
            return ps.tile([128, 2, 512], F32, tag="patt", bufs=2, name="patt")

        def pm():
            # prereq matmul outputs: independent 1-bank rotation (2 banks)
            return ps.tile([128, 512], F32, tag="pmisc", bufs=2, name="pmisc")

        # ---- constants ----
        ident = consts.tile([128, 128], F16)
        nc.gpsimd.memset(ident, 0.0)
        nc.gpsimd.affine_select(out=ident, in_=ident, compare_op=ALU.not_equal,
                                fill=1.0, base=0, pattern=[[-1, 128]],
                                channel_multiplier=1)
        ones_blk = consts.tile([128, 32], F16)
        nc.gpsimd.memset(ones_blk, 1.0)
        meanones = consts.tile([128, 1], F16)
        nc.gpsimd.memset(meanones, INV_N)
        negc = consts.tile([128, 1], F32)
        nc.gpsimd.memset(negc, -CLIP_C)
        ones_f32 = consts.tile([128, T], F32)
        nc.gpsimd.memset(ones_f32, 1.0)
        pidx_i = consts.tile([128, 4], I32)
        for nt in range(4):
            nc.gpsimd.iota(pidx_i[:, nt:nt + 1], pattern=[[0, 1]], base=128 * nt,
                           channel_multiplier=1)
        pidx_h = consts.tile([128, 4], F16)
        nc.vector.tensor_copy(pidx_h, pidx_i)

        # ---- weights to SBUF ----
        wnode_sb = wpool.tile([128, 2, 3 * D], F16)
        nc.sync.dma_start(wnode_sb, wnode_d.rearrange("(c p) j -> p c j", p=128))
        wfix_sb = wpool.tile([128, 2, D], F16)
        nc.sync.dma_start(wfix_sb, wfix_d.rearrange("(c p) j -> p c j", p=128))
        wstep_sb = wpool.tile([128, 2, D], F16)
        nc.sync.dma_start(wstep_sb, wstep_d.rearrange("(c p) j -> p c j", p=128))
        wout_sb = wpool.tile([128, 2, D], F16)
        nc.sync.dma_start(wout_sb, wout_d.rearrange("(c p) j -> p c j", p=128))

        for b in [bb for _ in range(repeat) for bb in range(b_loc)]:
            # ---- load batch data ----
            emb_nat = sb.tile([128, 4, D], F16, tag="emb_nat")
            nc.sync.dma_start(emb_nat, emb_d[b].rearrange("(o p) d -> p o d", p=128))
            c_bcast = sb.tile([128, T], F16, tag="c_bcast", bufs=2)
            nc.sync.dma_start(c_bcast, nodes_d[b][None, :].to_broadcast((128, T)))

            mask_u8 = sb.tile([128, 4, N], U8, tag="mask_u8", bufs=2)
            nc.sync.dma_start(mask_u8, mask_d[b].rearrange("(o p) n -> p o n", p=128))
            meng = nc.gpsimd if MGEN_ENG == "pool" else nc.vector
            mask01 = sb.tile([128, 4, N], F16, tag="mask01", bufs=3)
            meng.tensor_copy(mask01, mask_u8)
            invm = sb.tile([128, 4, N], F16, tag="invm", bufs=3)
            meng.tensor_scalar(invm, mask01, -1.0, 1.0, ALU.mult, ALU.add)

            # transposed mask, either as {0,1} (dve mode: multiply post-exp)
            # or as {-MCLIP, 0} (pe mode: add to scores pre-exp via PE)
            maskT = sb.tile([128, 4, T], F16, tag="maskT", bufs=2)
            for ntt in range(4):
                ptm = pm()
                for to in range(4):
                    nc.tensor.matmul(ptm[:, to * 128:(to + 1) * 128],
                                     mask01[:, to, ntt * 128:(ntt + 1) * 128],
                                     ident, start=True, stop=True)
                dst = maskT[:, ntt, :]
                if MASK_MODE == "pe":
                    nc.vector.tensor_scalar(dst, ptm, MCLIP, -MCLIP,
                                            ALU.mult, ALU.add)
                else:
                    nc.vector.tensor_copy(dst, ptm)

            # one-hot gather matrix S[n, t] = (n == current_nodes[t])
            s_oh = sb.tile([128, 4, T], F16, tag="s_oh", bufs=2)
            seng = nc.gpsimd if SOH_ENG == "pool" else nc.vector
            for ntt in range(4):
                seng.tensor_tensor(
                    s_oh[:, ntt, :], c_bcast,
                    pidx_h[:, ntt:ntt + 1].to_broadcast((128, T)), ALU.is_equal)

            # ---- embT via PE (identity matmul), one psum tile per d-chunk ----
            embT = sb.tile([128, 2, N], F16, tag="embT", bufs=2)
            for dc in range(2):
                pt = pm()
                for no in range(4):
                    nc.tensor.matmul(pt[:, no * 128:(no + 1) * 128],
                                     emb_nat[:, no, dc * 128:(dc + 1) * 128],
                                     ident, start=True, stop=True)
                nc.vector.tensor_copy(embT[:, dc, :], pt)

            # ---- graph context: gcT = W_fixed^T @ mean(emb) ----
            p_mean = pm()
            for dc in range(2):
                for no in range(4):
                    nc.tensor.matmul(p_mean[:, dc:dc + 1],
                                     emb_nat[:, no, dc * 128:(dc + 1) * 128],
                                     meanones, start=(no == 0), stop=(no == 3))
            meanT = sb.tile([128, 2], F16, tag="meanT")
            nc.vector.tensor_copy(meanT, p_mean[:, 0:2])
            p_gc = pm()
            for dpc in range(2):
                for dc in range(2):
                    nc.tensor.matmul(p_gc[:, dpc:dpc + 1],
                                     wfix_sb[:, dc, dpc * 128:(dpc + 1) * 128],
                                     meanT[:, dc:dc + 1],
                                     start=(dc == 0), stop=(dc == 1))
            gcT = sb.tile([128, 2], F32, tag="gcT")
            nc.vector.tensor_copy(gcT, p_gc[:, 0:2])

            # ---- kT / logit_kT: projT[j, n] = W^T emb^T ----
            kT = sb.tile([128, 2, N], F16, tag="kT", bufs=2)
            lkT = sb.tile([128, 2, N], F16, tag="lkT", bufs=3)
            for j, dst, jj in ((0, kT, 0), (1, kT, 1), (4, lkT, 0), (5, lkT, 1)):
                pp = pm()
                for dc in range(2):
                    nc.tensor.matmul(pp, wnode_sb[:, dc, j * 128:(j + 1) * 128],
                                     embT[:, dc, :], start=(dc == 0), stop=(dc == 1))
                nc.vector.tensor_copy(dst[:, jj, :], pp)

            # ---- v (natural layout, fp16) ----
            v_sb = sb.tile([128, 4, D], F16, tag="v_sb", bufs=2)
            for nv in range(2):
                pv = pm()
                for ni in range(2):
                    no = 2 * nv + ni
                    for dc in range(2):
                        nc.tensor.matmul(pv[:, ni * D:(ni + 1) * D],
                                         embT[:, dc, no * 128:(no + 1) * 128],
                                         wnode_sb[:, dc, D:2 * D],
                                         start=(dc == 0), stop=(dc == 1))
                nc.vector.tensor_copy(
                    v_sb[:, 2 * nv:2 * nv + 2, :],
                    pv.rearrange("p (c d) -> p c d", c=2))

            # ---- gathered step embeddings, transposed: seT = emb^T @ S ----
            seT = sb.tile([128, 2, T], F16, tag="seT", bufs=2)
            for dc in range(2):
                pse = pm()
                for no in range(4):
                    nc.tensor.matmul(pse,
                                     emb_nat[:, no, dc * 128:(dc + 1) * 128],
                                     s_oh[:, no, :], start=(no == 0), stop=(no == 3))
                nc.vector.tensor_copy(seT[:, dc, :], pse)

            # ---- qT = W_step^T @ seT + gcT ----
            qT = sb.tile([128, 2, T], F16, tag="qT", bufs=2)
            for dpc in range(2):
                pq = pm()
                for dc in range(2):
                    nc.tensor.matmul(pq,
                                     wstep_sb[:, dc, dpc * 128:(dpc + 1) * 128],
                                     seT[:, dc, :], start=(dc == 0), stop=(dc == 1))
                nc.vector.tensor_scalar(qT[:, dpc, :], pq,
                                        gcT[:, dpc:dpc + 1], None, ALU.add)

            # ---- multi-head attention, 2 groups of 4 heads ----
            hn = sb.tile([128, 2, T], F16, tag="hn", bufs=2)  # normalized headsT
            for g in range(2):
                e_sb = sb.tile([128, 4, 4, T], F16, tag="e_sb", bufs=2)
                p_hd = ptile([128, T], "pacc", 2)
                p_sm = ptile([128, T], "pacc", 2)
                for nt in range(4):
                    p_scs = []
                    for half in range(2):
                        p_sc = p2()
                        p_scs.append(p_sc)
                        if WARM:
                            # tiny dummy MM: keeps the PE HAM un-throttled
                            # through the exp wait; the real score MM below
                            # rewrites this region with start=True.
                            nc.tensor.matmul(p_sc[0:1, 0, 0:32],
                                             ident[:, 0:1], ones_blk,
                                             start=True, stop=True)
                        if MASK_MODE == "pe":
                            # additive {-MCLIP,0} mask first (full-array MMs),
                            # then the 4 score MMs run as a concurrent quad
                            for hh2 in range(2):
                                nc.tensor.matmul(
                                    p_sc[:, hh2, :], ident, maskT[:, nt, :],
                                    start=True, stop=False)
                    for half in range(2):
                        p_sc = p_scs[half]
                        for hh2 in range(2):
                            hh = 2 * half + hh2
                            nc.tensor.matmul(
                                p_sc[:, hh2, :],
                                kT[hh * 32:(hh + 1) * 32, g, nt * 128:(nt + 1) * 128],
                                qT[hh * 32:(hh + 1) * 32, g, :],
                                start=(MASK_MODE != "pe"), stop=True,
                                tile_position=(hh * 32, 0),
                                skip_group_check=(MASK_MODE == "pe"))
                    for half in range(2):
                        esl = e_sb[:, nt, 2 * half:2 * half + 2, :]
                        nc.scalar.activation(esl, p_scs[half], AF.Exp,
                                             scale=SCALE_DH)
                        if MASK_MODE != "pe":
                            nc.vector.tensor_tensor(
                                esl, esl,
                                maskT[:, nt, None, :].to_broadcast((128, 2, T)),
                                ALU.mult)
                    for hh in range(4):
                        h = 4 * g + hh
                        nc.tensor.matmul(
                            p_hd[hh * 32:(hh + 1) * 32, :],
                            v_sb[:, nt, h * 32:(h + 1) * 32],
                            e_sb[:, nt, hh, :],
                            start=(nt == 0), stop=(nt == 3),
                            tile_position=(0, hh * 32), skip_group_check=True)
                    for hh in range(4):
                        nc.tensor.matmul(
                            p_sm[hh * 32:(hh + 1) * 32, :],
                            ones_blk,
                            e_sb[:, nt, hh, :],
                            start=(nt == 0), stop=(nt == 3),
                            tile_position=(0, hh * 32), skip_group_check=True)
                recip = sb.tile([128, T], F32, tag="recip", bufs=2)
                if RECIP_MODE == "pool":
                    sm_sb = sb.tile([128, T], F32, tag="sm_sb", bufs=2)
                    nc.vector.tensor_copy(sm_sb, p_sm)
                    nc.gpsimd.tensor_tensor(recip, ones_f32, sm_sb, ALU.divide)
                elif RECIP_MODE == "exact":
                    nc.vector.reciprocal(out=recip, in_=p_sm)
                elif RECIP_MODE == "approx_sbuf":
                    sm_sb = sb.tile([128, T], F32, tag="sm_sb", bufs=2)
                    nc.vector.tensor_copy(sm_sb, p_sm)
                    nc.vector.reciprocal_approx_fast(out=recip, in_=sm_sb)
                else:
                    nc.vector.reciprocal_approx_fast(out=recip, in_=p_sm)
                nc.vector.tensor_tensor(hn[:, g, :], p_hd, recip, ALU.mult)

            # ---- glimpseT = W_out^T @ hn ----
            gT = sb.tile([128, 2, T], F16, tag="gT", bufs=2)
            for dpc in range(2):
                pg = pm()
                for dc in range(2):
                    nc.tensor.matmul(pg,
                                     wout_sb[:, dc, dpc * 128:(dpc + 1) * 128],
                                     hn[:, dc, :], start=(dc == 0), stop=(dc == 1))
                nc.vector.tensor_copy(gT[:, dpc, :], pg)

            # ---- pointer logits [t, n]; pipelined per logits-half:
            # tanh(tg) -> ein(tg) -> exps(tg) overlap tanh(tg+1)
            t1 = sb.tile([128, 4, N], F32, tag="t1", bufs=2)
            ein = sb.tile([128, 4, N], F32, tag="ein", bufs=2)
            invmB = sb.tile([128, 4, N], F32, tag="invmB", bufs=2)
            s2 = sb.tile([128, 4], F32, tag="s2", bufs=2)
            e2scr = sb.tile([128, N], F16, tag="e2scr")
            for tg in range(2):
                p_lg = p2()
                for tc2 in range(2):
                    tcn = 2 * tg + tc2
                    for dc in range(2):
                        nc.tensor.matmul(p_lg[:, tc2, :],
                                         gT[:, dc, tcn * 128:(tcn + 1) * 128],
                                         lkT[:, dc, :], start=(dc == 0), stop=(dc == 1))
                sl = slice(2 * tg, 2 * tg + 2)
                nc.scalar.activation(t1[:, sl, :], p_lg, AF.Tanh,
                                     scale=1.0 / np.sqrt(D))
                # ein = (t1 + C/10) * m; exp(10*ein - C): masked -> 0
                nc.vector.scalar_tensor_tensor(ein[:, sl, :], t1[:, sl, :],
                                               CLIP_C / 10.0, mask01[:, sl, :],
                                               ALU.add, ALU.mult)
                for tc2 in range(2):
                    tcn = 2 * tg + tc2
                    nc.scalar.activation(e2scr, ein[:, tcn, :], AF.Exp,
                                         scale=10.0, bias=negc[:, 0:1],
                                         accum_out=s2[:, tcn:tcn + 1])
                # invmB = invm*(BIG/10) - ein  (masked -> BIG/10, else -ein)
                nc.vector.scalar_tensor_tensor(invmB[:, sl, :], invm[:, sl, :],
                                               BIG / 10.0, ein[:, sl, :],
                                               ALU.mult, ALU.subtract)

            # ---- Lp = (ln(s2) + C)/10 on DVE (bit trick + poly) ----
            s2i = s2.bitcast(I32)
            e_i = sb.tile([128, 4], I32, tag="e_i", bufs=2)
            nc.vector.tensor_scalar(e_i, s2i, 23, None, ALU.arith_shift_right)
            e_f = sb.tile([128, 4], F32, tag="e_f", bufs=2)
            nc.vector.tensor_scalar(e_f, e_i, -127, LN2 / 10.0, ALU.add, ALU.mult)
            m_i = sb.tile([128, 4], I32, tag="m_i", bufs=2)
            nc.vector.tensor_scalar(m_i, s2i, 0x007FFFFF, 0x3F800000,
                                    ALU.bitwise_and, ALU.bitwise_or)
            m_f = m_i.bitcast(F32)
            q5 = sb.tile([128, 4], F32, tag="q5", bufs=2)
            nc.vector.scalar_tensor_tensor(q5, m_f, LC[4] / LC[5], m_f,
                                           ALU.add, ALU.mult)
            nc.vector.scalar_tensor_tensor(q5, q5, LC[3] / LC[5], m_f,
                                           ALU.add, ALU.mult)
            nc.vector.scalar_tensor_tensor(q5, q5, LC[2] / LC[5], m_f,
                                           ALU.add, ALU.mult)
            nc.vector.scalar_tensor_tensor(q5, q5, LC[1] / LC[5], m_f,
                                           ALU.add, ALU.mult)
            lp = sb.tile([128, 4], F32, tag="lp", bufs=2)
            nc.vector.scalar_tensor_tensor(lp, q5, LC[5] / 10.0, e_f,
                                           ALU.mult, ALU.add)
            nc.vector.tensor_scalar(lp, lp, (LC[0] + CLIP_C) / 10.0, None, ALU.add)

            # ---- final: out = -10*(invmB + Lp) ----
            for tcn in range(4):
                o = sb.tile([128, N], F32, tag="o_out")
                nc.vector.tensor_scalar(o, invmB[:, tcn, :],
                                        lp[:, tcn:tcn + 1], -10.0,
                                        ALU.add, ALU.mult)
                nc.sync.dma_start(out_d[b, tcn * 128:(tcn + 1) * 128, :], o)


_NC_CACHE = {}


def kernel(**inputs):
    emb = np.ascontiguousarray(np.asarray(inputs["embeddings"]).astype(np.float16))
    nodes = np.ascontiguousarray(np.asarray(inputs["current_nodes"]).astype(np.float16))
    mask = np.ascontiguousarray(np.asarray(inputs["action_mask"]).astype(np.uint8))
    w_node = np.ascontiguousarray(np.asarray(inputs["W_node"]).astype(np.float16))
    w_fixed = np.ascontiguousarray(np.asarray(inputs["W_fixed"]).astype(np.float16))
    w_step = np.ascontiguousarray(np.asarray(inputs["W_step"]).astype(np.float16))
    w_out = np.ascontiguousarray(np.asarray(inputs["W_out"]).astype(np.float16))

    if "nc" not in _NC_CACHE:
        _NC_CACHE["nc"] = build_kernel()
    nc = _NC_CACHE["nc"]

    in_maps = []
    for c in range(N_CORES):
        lo, hi = c * B_LOC, (c + 1) * B_LOC
        in_maps.append({
            "embeddings": emb[lo:hi],
            "current_nodes": nodes[lo:hi],
            "action_mask": mask[lo:hi],
            "W_node": w_node,
            "W_fixed": w_fixed,
            "W_step": w_step,
            "W_out": w_out,
        })
    res = run_bass_kernel_spmd(nc, in_maps, list(range(N_CORES)))
    out = np.concatenate([res.results[c]["log_p"] for c in range(N_CORES)], axis=0)
    return out.astype(np.float32)


if __name__ == "__main__":
    import reference
    inputs = {k: np.asarray(v) for k, v in reference.setup_inputs().items()}
    expected = np.asarray(reference.reference(**inputs))
    actual = kernel(**inputs)
    err = np.abs(actual - expected)
    denom = np.maximum(np.abs(expected), 1e-6)
    print("max abs err:", err.max())
    print("max rel err:", (err / denom).max())

